# revision 28
# baseline (speedup 1.0000x reference)
"""Trainium2 Bass kernel for additive-attention pooling.

Computation (per batch row b):
    Wah   = h @ Wah_w.T                         [B, HID]
    e     = tanh(Wah[:, None, :] + p_att_feats) [B, L, HID]
    s     = e @ alpha_w[0]                      [B, L]
    alpha = softmax(s, -1)                      [B, L]
    att   = sum_l alpha[b, l] * att_feats[b, l, :]   [B, FEAT]

Sharding: pure data parallel over the batch dim, 32 rows per core on 8
NeuronCores; the small Wah_w / alpha_w weights are replicated.

Per-core dataflow (DMA-bound, ~64 MB of input per core):
  setup : PE-transpose h and Wah_w, compute WahT[h, b]; transpose alpha_w.
  phase1: stream p_att[b] -> PE transpose to [h, l] -> ScalarE fused
          bias(=Wah)+tanh (bf16 out) -> TensorE contracts h with alpha_w
          -> scores[1, L] -> ScalarE exp with fused row-sum -> reciprocal
          -> two K=1 matmuls transpose alpha row into a [L, 1] column,
          folding the 1/sum normalization into the matmul's rhs scalar.
  phase2: stream att_feats[b] as [l, f] tiles -> per batch 8 matvec
          matmuls (float32r, full-rate fp32 streaming) accumulate
          att[1, 512] per PSUM bank -> copy to staging -> one output DMA.
"""

import os
import sys
import types

sys.path.insert(0, "/opt/trn_rl_repo")

# This image's antenv package lacks axon_hooks; provide it so
# concourse.bass_utils can import it (trace path) without crashing.
if "antenv.axon_hooks" not in sys.modules:
    _m = types.ModuleType("antenv.axon_hooks")

    def _set_hook(h):
        _m._hook = h

    def _get_hook():
        return getattr(_m, "_hook", None)

    _m.set_axon_ntff_profile_hook = _set_hook
    _m.get_axon_ntff_profile_hook = _get_hook
    sys.modules["antenv.axon_hooks"] = _m
    import antenv

    antenv.axon_hooks = _m

import numpy as np  # noqa: E402
import bass_rust  # noqa: E402
import concourse.bass as bass  # noqa: E402
import concourse.tile as tile  # noqa: E402
from concourse import mybir  # noqa: E402
from concourse.masks import make_identity  # noqa: E402
from concourse.tile_rust import add_dep_helper  # noqa: E402

F32 = mybir.dt.float32
F32R = mybir.dt.float32r
BF16 = mybir.dt.bfloat16
PSUM = bass.MemorySpace.PSUM
Tanh = mybir.ActivationFunctionType.Tanh
Exp = mybir.ActivationFunctionType.Exp

B, L, RNN, HID, FEAT = 256, 196, 1024, 512, 2048
NCORES = 8
BL = B // NCORES  # batch rows per core
L_HI = 128
L_LO = L - L_HI  # 68
NHC = HID // 128  # h chunks
NRC = RNN // 128  # r chunks
NFQ = FEAT // 512  # psum-bank-sized f chunks

# float32r streams the rhs at 1 cycle/row (vs 4 for float32) when N >= 256.
USE_F32R = os.environ.get("KERNEL_NO_F32R", "") != "1"
AF_BUFS = int(os.environ.get("KERNEL_AF_BUFS", "3"))


def _split_sync(nc):
    """walrus in this image encodes at most ONE semaphore wait and ONE
    semaphore update per instruction; Tile freely emits several. Move the
    extras onto single-wait/single-update NoOp carriers on the same engine
    (engine queues are strict FIFO, so a preceding NoOp's wait gates the
    instruction and a following NoOp's update fires after it completes)."""
    dma_types = {
        "InstDMACopy",
        "InstTensorLoad",
        "InstTensorSave",
        "InstDmaTransposeAnt",
        "InstTensorCopy",
    }
    for f in nc.m.functions:
        for bb in f.blocks:
            new = []
            changed = False
            for ins in bb.instructions:
                si = ins.sync_info
                if si is None:
                    new.append(ins)
                    continue
                waits = list(si.on_wait)
                updates = list(si.on_update)
                if len(waits) <= 1 and len(updates) <= 1:
                    new.append(ins)
                    continue
                changed = True
                tname = type(ins).__name__
                for j, w in enumerate(waits[:-1]):
                    nop = mybir.InstNoOp(name=f"{ins.name}_w{j}", ins=[], outs=[])
                    nop.engine = ins.engine
                    nop.sync_info = bass_rust.SyncInfo(on_wait=[w], on_update=[])
                    new.append(nop)
                keep_w = waits[-1:]
                post_u = []
                keep_u = updates
                if len(updates) > 1:
                    if tname in dma_types:
                        raise RuntimeError(
                            f"DMA instruction {ins.name} carries {len(updates)} "
                            "sem updates; cannot split without changing semantics"
                        )
                    keep_u = updates[:1]
                    post_u = updates[1:]
                ins.sync_info = bass_rust.SyncInfo(on_wait=keep_w, on_update=keep_u)
                new.append(ins)
                for j, u in enumerate(post_u):
                    nop = mybir.InstNoOp(name=f"{ins.name}_u{j}", ins=[], outs=[])
                    nop.engine = ins.engine
                    nop.sync_info = bass_rust.SyncInfo(on_wait=[], on_update=[u])
                    new.append(nop)
            if changed:
                bb.instructions = new


def build_nc(split=True):
    nc = bass.Bass()
    mm_dt = F32R if USE_F32R else F32
    h_d = nc.declare_dram_parameter("h", [BL, RNN], F32, isOutput=False)
    af_d = nc.declare_dram_parameter("att_feats", [BL, L, FEAT], mm_dt, isOutput=False)
    pa_d = nc.declare_dram_parameter("p_att_feats", [BL, L, HID], F32, isOutput=False)
    ww_d = nc.declare_dram_parameter("Wah_w", [HID, RNN], F32, isOutput=False)
    aw_d = nc.declare_dram_parameter("alpha_w", [1, HID], F32, isOutput=False)
    out_d = nc.declare_dram_parameter("out", [BL, FEAT], F32, isOutput=True)

    with tile.TileContext(nc) as tc:
        with tc.tile_pool(name="singles", bufs=1) as singles:
            identity = singles.tile([128, 128], F32)
            make_identity(nc, identity[:])
            wahT = singles.tile([128, NHC, BL], F32)  # WahT[h % 128, hc, b]
            awT = singles.tile([128, NHC], BF16)  # alpha_w^T chunks
            # exp(scores): 256-wide zero-padded slot per batch so the two
            # alphaT transpose matmuls both span 128 output partitions (the
            # PSUM accumulation-group bookkeeping is per-partition)
            LP = 256
            expS = singles.tile([1, BL * LP], F32)
            nc.gpsimd.memset(expS[:], 0.0)
            sums = singles.tile([1, BL], F32)
            rsum = singles.tile([1, BL], F32)
            aT_sb = singles.tile([128, BL, 2], mm_dt)  # alphaT cols (hi, lo)

            # ---------------- setup: weights ----------------
            with (
                tc.tile_pool(name="setup_sb", bufs=1) as ssb,
                tc.tile_pool(name="setup_ps", bufs=2, space=PSUM) as sps,
            ):
                h_sb = ssb.tile([BL, RNN], F32)
                nc.sync.dma_start(h_sb[:], h_d[:])
                ww_sb = ssb.tile([128, NHC, RNN], F32)
                nc.sync.dma_start(
                    ww_sb[:], ww_d[:].rearrange("(c p) r -> p c r", p=128)
                )
                aw_sb = ssb.tile([1, HID], F32)
                nc.sync.dma_start(aw_sb[:], aw_d[:])
                ones11 = ssb.tile([1, 1], F32)
                nc.gpsimd.memset(ones11[:], 1.0)

                # h^T: [r % 128, rc, b]
                hT = ssb.tile([128, NRC, BL], F32)
                for rc in range(NRC):
                    ps = sps.tile([128, BL], F32, tag="t_small")
                    nc.tensor.transpose(
                        ps[:], h_sb[:, rc * 128 : (rc + 1) * 128], identity[:BL, :BL]
                    )
                    nc.vector.tensor_copy(hT[:, rc, :], ps[:])

                # Wah_w^T: [r % 128, rc, h]
                wwT = ssb.tile([128, NRC, HID], F32)
                for rc in range(NRC):
                    for hc in range(NHC):
                        ps = sps.tile([128, 128], F32, tag="t_big")
                        nc.tensor.transpose(
                            ps[:],
                            ww_sb[:, hc, rc * 128 : (rc + 1) * 128],
                            identity[:],
                        )
                        nc.vector.tensor_copy(
                            wwT[:, rc, hc * 128 : (hc + 1) * 128], ps[:]
                        )

                # WahT[h, b] = sum_r Wah_w[h, r] * h[b, r]
                for hc in range(NHC):
                    ps = sps.tile([128, BL], F32, tag="mm")
                    for rc in range(NRC):
                        nc.tensor.matmul(
                            ps[:],
                            wwT[:, rc, hc * 128 : (hc + 1) * 128],
                            hT[:, rc, :],
                            start=(rc == 0),
                            stop=(rc == NRC - 1),
                        )
                    nc.vector.tensor_copy(wahT[:, hc, :], ps[:])

                # alpha_w^T columns (bf16 to match bf16 e tiles)
                for hc in range(NHC):
                    ps = sps.tile([128, 1], F32, tag="aw")
                    nc.tensor.matmul(
                        ps[:],
                        aw_sb[0:1, hc * 128 : (hc + 1) * 128],
                        ones11[:],
                        start=True,
                        stop=True,
                    )
                    nc.vector.tensor_copy(awT[:, hc : hc + 1], ps[:])

            # ---------------- streaming batch loop ----------------
            with (
                tc.tile_pool(name="pa", bufs=3) as pool_pa,
                tc.tile_pool(name="af", bufs=AF_BUFS) as pool_af,
                tc.tile_pool(name="e", bufs=6) as pool_e,
                tc.tile_pool(name="tp_ps", bufs=2, space=PSUM) as pool_tp,
                tc.tile_pool(name="sc_ps", bufs=2, space=PSUM) as pool_sc,
                tc.tile_pool(name="aT_ps", bufs=2, space=PSUM) as pool_aT,
                tc.tile_pool(name="ao_ps", bufs=2, space=PSUM) as pool_ao,
                tc.tile_pool(name="ob", bufs=2) as pool_ob,
            ):
                prev_aT_read = None
                for p in range(BL // 2):
                    b0 = 2 * p
                    pa_hi = pool_pa.tile([L_HI, 2, HID], F32, tag="pa_hi")
                    nc.sync.dma_start(
                        pa_hi[:],
                        pa_d[b0 : b0 + 2, 0:L_HI, :].rearrange("b l c -> l b c"),
                    )
                    # 68-partition DMAs fan out to only 4 SDMA engines (engine
                    # count = largest power of two dividing P, capped at 16);
                    # split 68 -> 64 + 4 to spread across all 16 engines.
                    pa_lo = pool_pa.tile([L_LO, 2, HID], F32, tag="pa_lo")
                    nc.sync.dma_start(
                        pa_lo[0:64],
                        pa_d[b0 : b0 + 2, L_HI : L_HI + 64, :].rearrange(
                            "b l c -> l b c"
                        ),
                    )
                    nc.sync.dma_start(
                        pa_lo[64:L_LO],
                        pa_d[b0 : b0 + 2, L_HI + 64 : L, :].rearrange(
                            "b l c -> l b c"
                        ),
                    )
                    # split each 8 KB feature row into two 4 KB descriptors —
                    # the SDMA per-descriptor sweet spot (8 KB rows measured
                    # 457 ns vs 188 ns per 4 KB)
                    af_hi = pool_af.tile([L_HI, 2, FEAT], mm_dt, tag="af_hi")
                    af_lo = pool_af.tile([L_LO, 2, FEAT], mm_dt, tag="af_lo")
                    FH = FEAT // 2
                    for fh in range(2):
                        fs = slice(fh * FH, (fh + 1) * FH)
                        nc.sync.dma_start(
                            af_hi[:, :, fs],
                            af_d[b0 : b0 + 2, 0:L_HI, fs].rearrange(
                                "b l c -> l b c"
                            ),
                        )
                        nc.sync.dma_start(
                            af_lo[0:64, :, fs],
                            af_d[b0 : b0 + 2, L_HI : L_HI + 64, fs].rearrange(
                                "b l c -> l b c"
                            ),
                        )
                        nc.sync.dma_start(
                            af_lo[64:L_LO, :, fs],
                            af_d[b0 : b0 + 2, L_HI + 64 : L, fs].rearrange(
                                "b l c -> l b c"
                            ),
                        )

                    # output rows for this pair, staged flat on partition 0
                    ob = pool_ob.tile([1, 2 * FEAT], F32)

                    # -------- phase 1: scores for both batches of the pair --------
                    sc = pool_sc.tile([1, 2, L], F32)
                    for hc in range(NHC):
                        hsl = slice(hc * 128, (hc + 1) * 128)
                        e_bf = pool_e.tile([128, 2, L], BF16)
                        for jb in range(2):
                            b = b0 + jb
                            tp = pool_tp.tile([128, L], F32)
                            t1 = nc.tensor.matmul(
                                tp[:, 0:L_HI],
                                pa_hi[:, jb, hsl],
                                identity[:],
                                is_transpose=True,
                                start=True,
                                stop=False,
                            )
                            t2 = nc.tensor.matmul(
                                tp[:, L_HI:L],
                                pa_lo[:, jb, hsl],
                                identity[:L_LO, :L_LO],
                                is_transpose=True,
                                start=False,
                                stop=True,
                            )
                            add_dep_helper(t2.ins, t1.ins, sync=False, reason="tpord")
                            nc.scalar.activation(
                                e_bf[:, jb, :], tp[:], Tanh, bias=wahT[:, hc, b : b + 1]
                            )
                        nc.tensor.matmul(
                            sc[:],
                            awT[:, hc : hc + 1],
                            e_bf[:],
                            start=(hc == 0),
                            stop=(hc == NHC - 1),
                        )

                    for jb in range(2):
                        b = b0 + jb
                        # exp with fused row-sum, then 1/sum
                        nc.scalar.activation(
                            expS[0:1, b * LP : b * LP + L],
                            sc[0:1, jb, :],
                            Exp,
                            accum_out=sums[0:1, b : b + 1],
                        )
                        nc.vector.reciprocal(
                            rsum[0:1, b : b + 1], sums[0:1, b : b + 1]
                        )

                        # alphaT columns via K=1 matmuls; rhs=1/sum normalizes
                        aT = pool_aT.tile([128, 2], F32)
                        if prev_aT_read is not None:
                            # bufs=1: this start=True reopens the bank; it must
                            # wait for the previous batch's col-1 read (regions
                            # are disjoint, so Tile tracks no dep itself)
                            pre_m1 = prev_aT_read
                        else:
                            pre_m1 = None
                        m1 = nc.tensor.matmul(
                            aT[:, 0:1],
                            expS[0:1, b * LP : b * LP + 128],
                            rsum[0:1, b : b + 1],
                            start=True,
                            stop=False,
                        )
                        m2 = nc.tensor.matmul(
                            aT[:, 1:2],
                            expS[0:1, b * LP + 128 : b * LP + 256],
                            rsum[0:1, b : b + 1],
                            start=False,
                            stop=True,
                        )
                        add_dep_helper(m2.ins, m1.ins, sync=False, reason="aTord")
                        if pre_m1 is not None:
                            add_dep_helper(m1.ins, pre_m1, sync=True, reason="aTwar")
                        c1 = nc.vector.tensor_copy(aT_sb[:, b, 0:1], aT[:, 0:1])
                        # col-0 read must wait until the accumulation group
                        # (closed by m2) is complete
                        add_dep_helper(c1.ins, m2.ins, sync=True, reason="aTgrp")
                        c2 = nc.vector.tensor_copy(
                            aT_sb[0:L_LO, b, 1:2], aT[0:L_LO, 1:2]
                        )
                        prev_aT_read = c2.ins

                        # -------- phase 2: weighted sum of att_feats --------
                        for q in range(NFQ):
                            fsl = slice(q * 512, (q + 1) * 512)
                            ao = pool_ao.tile([1, 512], F32)
                            nc.tensor.matmul(
                                ao[:],
                                aT_sb[:, b, 0:1],
                                af_hi[:, jb, fsl],
                                start=True,
                                stop=False,
                            )
                            nc.tensor.matmul(
                                ao[:],
                                aT_sb[0:L_LO, b, 1:2],
                                af_lo[:, jb, fsl],
                                start=False,
                                stop=True,
                            )
                            osl = slice(
                                jb * FEAT + q * 512, jb * FEAT + (q + 1) * 512
                            )
                            if q % 2 == 0:
                                nc.vector.tensor_copy(ob[0:1, osl], ao[:])
                            else:
                                nc.scalar.copy(ob[0:1, osl], ao[:])

                    nc.sync.dma_start(out_d[b0 : b0 + 2, :], ob[:])

    if split:
        _split_sync(nc)
    return nc


_NC_CACHE = None


def _get_nc():
    global _NC_CACHE
    if _NC_CACHE is None:
        _NC_CACHE = build_nc()
    return _NC_CACHE


def _make_in_maps(h, att_feats, p_att_feats, Wah_w, alpha_w):
    h = np.ascontiguousarray(h, dtype=np.float32)
    att_feats = np.ascontiguousarray(att_feats, dtype=np.float32)
    p_att_feats = np.ascontiguousarray(p_att_feats, dtype=np.float32)
    Wah_w = np.ascontiguousarray(Wah_w, dtype=np.float32)
    alpha_w = np.ascontiguousarray(alpha_w, dtype=np.float32)
    in_maps = []
    for i in range(NCORES):
        sl = slice(i * BL, (i + 1) * BL)
        in_maps.append(
            {
                "h": np.ascontiguousarray(h[sl]),
                "att_feats": np.ascontiguousarray(att_feats[sl]),
                "p_att_feats": np.ascontiguousarray(p_att_feats[sl]),
                "Wah_w": Wah_w,
                "alpha_w": alpha_w,
            }
        )
    return in_maps


def run_spmd(h, att_feats, p_att_feats, Wah_w, alpha_w, trace=False):
    """Run the SPMD kernel; returns (full_output, BassKernelResults)."""
    from concourse.bass_utils import run_bass_kernel_spmd

    nc = _get_nc()
    in_maps = _make_in_maps(h, att_feats, p_att_feats, Wah_w, alpha_w)
    res = run_bass_kernel_spmd(nc, in_maps, list(range(NCORES)), trace=trace)
    out = np.concatenate([res.results[i]["out"] for i in range(NCORES)], axis=0)
    return out, res


def kernel(h, att_feats, p_att_feats, Wah_w, alpha_w):
    out, _ = run_spmd(h, att_feats, p_att_feats, Wah_w, alpha_w, trace=False)
    return out


# revision 30
# speedup vs baseline: 1.0635x; 1.0635x over previous
"""Trainium2 Bass kernel for additive-attention pooling.

Computation (per batch row b):
    Wah   = h @ Wah_w.T                         [B, HID]
    e     = tanh(Wah[:, None, :] + p_att_feats) [B, L, HID]
    s     = e @ alpha_w[0]                      [B, L]
    alpha = softmax(s, -1)                      [B, L]
    att   = sum_l alpha[b, l] * att_feats[b, l, :]   [B, FEAT]

Sharding: pure data parallel over the batch dim, 32 rows per core on 8
NeuronCores; the small Wah_w / alpha_w weights are replicated.

Per-core dataflow (DMA-bound, ~64 MB of input per core):
  setup : PE-transpose h and Wah_w, compute WahT[h, b]; transpose alpha_w.
  phase1: stream p_att[b] -> PE transpose to [h, l] -> ScalarE fused
          bias(=Wah)+tanh (bf16 out) -> TensorE contracts h with alpha_w
          -> scores[1, L] -> ScalarE exp with fused row-sum -> reciprocal
          -> two K=1 matmuls transpose alpha row into a [L, 1] column,
          folding the 1/sum normalization into the matmul's rhs scalar.
  phase2: stream att_feats[b] as [l, f] tiles -> per batch 8 matvec
          matmuls (float32r, full-rate fp32 streaming) accumulate
          att[1, 512] per PSUM bank -> copy to staging -> one output DMA.
"""

import os
import sys
import types

sys.path.insert(0, "/opt/trn_rl_repo")

# This image's antenv package lacks axon_hooks; provide it so
# concourse.bass_utils can import it (trace path) without crashing.
if "antenv.axon_hooks" not in sys.modules:
    _m = types.ModuleType("antenv.axon_hooks")

    def _set_hook(h):
        _m._hook = h

    def _get_hook():
        return getattr(_m, "_hook", None)

    _m.set_axon_ntff_profile_hook = _set_hook
    _m.get_axon_ntff_profile_hook = _get_hook
    sys.modules["antenv.axon_hooks"] = _m
    import antenv

    antenv.axon_hooks = _m

import numpy as np  # noqa: E402
import bass_rust  # noqa: E402
import concourse.bass as bass  # noqa: E402
import concourse.tile as tile  # noqa: E402
from concourse import mybir  # noqa: E402
from concourse.masks import make_identity  # noqa: E402
from concourse.tile_rust import add_dep_helper  # noqa: E402

F32 = mybir.dt.float32
F32R = mybir.dt.float32r
BF16 = mybir.dt.bfloat16
PSUM = bass.MemorySpace.PSUM
Tanh = mybir.ActivationFunctionType.Tanh
Exp = mybir.ActivationFunctionType.Exp

B, L, RNN, HID, FEAT = 256, 196, 1024, 512, 2048
NCORES = 8
BL = B // NCORES  # batch rows per core
L_HI = 128
L_LO = L - L_HI  # 68
NHC = HID // 128  # h chunks
NRC = RNN // 128  # r chunks
NFQ = FEAT // 512  # psum-bank-sized f chunks

# float32r streams the rhs at 1 cycle/row (vs 4 for float32) when N >= 256.
USE_F32R = os.environ.get("KERNEL_NO_F32R", "") != "1"
AF_BUFS = int(os.environ.get("KERNEL_AF_BUFS", "3"))


def _split_sync(nc):
    """walrus in this image encodes at most ONE semaphore wait and ONE
    semaphore update per instruction; Tile freely emits several. Move the
    extras onto single-wait/single-update NoOp carriers on the same engine
    (engine queues are strict FIFO, so a preceding NoOp's wait gates the
    instruction and a following NoOp's update fires after it completes)."""
    dma_types = {
        "InstDMACopy",
        "InstTensorLoad",
        "InstTensorSave",
        "InstDmaTransposeAnt",
        "InstTensorCopy",
    }
    for f in nc.m.functions:
        for bb in f.blocks:
            new = []
            changed = False
            for ins in bb.instructions:
                si = ins.sync_info
                if si is None:
                    new.append(ins)
                    continue
                waits = list(si.on_wait)
                updates = list(si.on_update)
                if len(waits) <= 1 and len(updates) <= 1:
                    new.append(ins)
                    continue
                changed = True
                tname = type(ins).__name__
                for j, w in enumerate(waits[:-1]):
                    nop = mybir.InstNoOp(name=f"{ins.name}_w{j}", ins=[], outs=[])
                    nop.engine = ins.engine
                    nop.sync_info = bass_rust.SyncInfo(on_wait=[w], on_update=[])
                    new.append(nop)
                keep_w = waits[-1:]
                post_u = []
                keep_u = updates
                if len(updates) > 1:
                    if tname in dma_types:
                        raise RuntimeError(
                            f"DMA instruction {ins.name} carries {len(updates)} "
                            "sem updates; cannot split without changing semantics"
                        )
                    keep_u = updates[:1]
                    post_u = updates[1:]
                ins.sync_info = bass_rust.SyncInfo(on_wait=keep_w, on_update=keep_u)
                new.append(ins)
                for j, u in enumerate(post_u):
                    nop = mybir.InstNoOp(name=f"{ins.name}_u{j}", ins=[], outs=[])
                    nop.engine = ins.engine
                    nop.sync_info = bass_rust.SyncInfo(on_wait=[], on_update=[u])
                    new.append(nop)
            if changed:
                bb.instructions = new


def build_nc(split=True):
    nc = bass.Bass()
    mm_dt = F32R if USE_F32R else F32
    h_d = nc.declare_dram_parameter("h", [BL, RNN], F32, isOutput=False)
    af_d = nc.declare_dram_parameter("att_feats", [BL, L, FEAT], mm_dt, isOutput=False)
    pa_d = nc.declare_dram_parameter("p_att_feats", [BL, L, HID], F32, isOutput=False)
    ww_d = nc.declare_dram_parameter("Wah_w", [HID, RNN], F32, isOutput=False)
    aw_d = nc.declare_dram_parameter("alpha_w", [1, HID], F32, isOutput=False)
    out_d = nc.declare_dram_parameter("out", [BL, FEAT], F32, isOutput=True)

    with tile.TileContext(nc) as tc:
        with tc.tile_pool(name="singles", bufs=1) as singles:
            identity = singles.tile([128, 128], F32)
            make_identity(nc, identity[:])
            wahT = singles.tile([128, NHC, BL], F32)  # WahT[h % 128, hc, b]
            awT = singles.tile([128, NHC], BF16)  # alpha_w^T chunks
            # exp(scores): 256-wide zero-padded slot per batch so the two
            # alphaT transpose matmuls both span 128 output partitions (the
            # PSUM accumulation-group bookkeeping is per-partition)
            LP = 256
            expS = singles.tile([1, BL * LP], F32)
            nc.gpsimd.memset(expS[:], 0.0)
            sums = singles.tile([1, BL], F32)
            rsum = singles.tile([1, BL], F32)
            aT_sb = singles.tile([128, BL, 2], mm_dt)  # alphaT cols (hi, lo)

            # ---------------- setup: weights ----------------
            with (
                tc.tile_pool(name="setup_sb", bufs=1) as ssb,
                tc.tile_pool(name="setup_ps", bufs=2, space=PSUM) as sps,
            ):
                h_sb = ssb.tile([BL, RNN], F32)
                nc.sync.dma_start(h_sb[:], h_d[:])
                ww_sb = ssb.tile([128, NHC, RNN], F32)
                nc.sync.dma_start(
                    ww_sb[:], ww_d[:].rearrange("(c p) r -> p c r", p=128)
                )
                aw_sb = ssb.tile([1, HID], F32)
                nc.sync.dma_start(aw_sb[:], aw_d[:])
                ones11 = ssb.tile([1, 1], F32)
                nc.gpsimd.memset(ones11[:], 1.0)

                # h^T: [r % 128, rc, b]
                hT = ssb.tile([128, NRC, BL], F32)
                for rc in range(NRC):
                    ps = sps.tile([128, BL], F32, tag="t_small")
                    nc.tensor.transpose(
                        ps[:], h_sb[:, rc * 128 : (rc + 1) * 128], identity[:BL, :BL]
                    )
                    nc.vector.tensor_copy(hT[:, rc, :], ps[:])

                # Wah_w^T: [r % 128, rc, h]
                wwT = ssb.tile([128, NRC, HID], F32)
                for rc in range(NRC):
                    for hc in range(NHC):
                        ps = sps.tile([128, 128], F32, tag="t_big")
                        nc.tensor.transpose(
                            ps[:],
                            ww_sb[:, hc, rc * 128 : (rc + 1) * 128],
                            identity[:],
                        )
                        nc.vector.tensor_copy(
                            wwT[:, rc, hc * 128 : (hc + 1) * 128], ps[:]
                        )

                # WahT[h, b] = sum_r Wah_w[h, r] * h[b, r]
                for hc in range(NHC):
                    ps = sps.tile([128, BL], F32, tag="mm")
                    for rc in range(NRC):
                        nc.tensor.matmul(
                            ps[:],
                            wwT[:, rc, hc * 128 : (hc + 1) * 128],
                            hT[:, rc, :],
                            start=(rc == 0),
                            stop=(rc == NRC - 1),
                        )
                    nc.vector.tensor_copy(wahT[:, hc, :], ps[:])

                # alpha_w^T columns (bf16 to match bf16 e tiles)
                for hc in range(NHC):
                    ps = sps.tile([128, 1], F32, tag="aw")
                    nc.tensor.matmul(
                        ps[:],
                        aw_sb[0:1, hc * 128 : (hc + 1) * 128],
                        ones11[:],
                        start=True,
                        stop=True,
                    )
                    nc.vector.tensor_copy(awT[:, hc : hc + 1], ps[:])

            # ---------------- streaming batch loop ----------------
            with (
                tc.tile_pool(name="pa", bufs=3) as pool_pa,
                tc.tile_pool(name="af", bufs=AF_BUFS) as pool_af,
                tc.tile_pool(name="e", bufs=6) as pool_e,
                tc.tile_pool(name="tp_ps", bufs=3, space=PSUM) as pool_tp,
                tc.tile_pool(name="sc_ps", bufs=2, space=PSUM) as pool_sc,
                tc.tile_pool(name="aT_ps", bufs=1, space=PSUM) as pool_aT,
                tc.tile_pool(name="ao_ps", bufs=2, space=PSUM) as pool_ao,
                tc.tile_pool(name="ob", bufs=2) as pool_ob,
            ):
                prev_aT_read = None
                for p in range(BL // 2):
                    b0 = 2 * p
                    pa_hi = pool_pa.tile([L_HI, 2, HID], F32, tag="pa_hi")
                    nc.sync.dma_start(
                        pa_hi[:],
                        pa_d[b0 : b0 + 2, 0:L_HI, :].rearrange("b l c -> l b c"),
                    )
                    # 68-partition DMAs fan out to only 4 SDMA engines (engine
                    # count = largest power of two dividing P, capped at 16);
                    # split 68 -> 64 + 4 to spread across all 16 engines.
                    pa_lo = pool_pa.tile([L_LO, 2, HID], F32, tag="pa_lo")
                    nc.sync.dma_start(
                        pa_lo[0:64],
                        pa_d[b0 : b0 + 2, L_HI : L_HI + 64, :].rearrange(
                            "b l c -> l b c"
                        ),
                    )
                    nc.sync.dma_start(
                        pa_lo[64:L_LO],
                        pa_d[b0 : b0 + 2, L_HI + 64 : L, :].rearrange(
                            "b l c -> l b c"
                        ),
                    )
                    af_hi = pool_af.tile([L_HI, 2, FEAT], mm_dt, tag="af_hi")
                    af_lo = pool_af.tile([L_LO, 2, FEAT], mm_dt, tag="af_lo")
                    nc.sync.dma_start(
                        af_hi[:],
                        af_d[b0 : b0 + 2, 0:L_HI, :].rearrange("b l c -> l b c"),
                    )
                    nc.sync.dma_start(
                        af_lo[0:64],
                        af_d[b0 : b0 + 2, L_HI : L_HI + 64, :].rearrange(
                            "b l c -> l b c"
                        ),
                    )
                    nc.sync.dma_start(
                        af_lo[64:L_LO],
                        af_d[b0 : b0 + 2, L_HI + 64 : L, :].rearrange(
                            "b l c -> l b c"
                        ),
                    )

                    # output rows for this pair, staged flat on partition 0
                    ob = pool_ob.tile([1, 2 * FEAT], F32)

                    # -------- phase 1: scores for both batches of the pair --------
                    sc = pool_sc.tile([1, 2, L], F32)
                    for hc in range(NHC):
                        hsl = slice(hc * 128, (hc + 1) * 128)
                        e_bf = pool_e.tile([128, 2, L], BF16)
                        for jb in range(2):
                            b = b0 + jb
                            tp = pool_tp.tile([128, L], F32)
                            t1 = nc.tensor.matmul(
                                tp[:, 0:L_HI],
                                pa_hi[:, jb, hsl],
                                identity[:],
                                is_transpose=True,
                                start=True,
                                stop=False,
                            )
                            t2 = nc.tensor.matmul(
                                tp[:, L_HI:L],
                                pa_lo[:, jb, hsl],
                                identity[:L_LO, :L_LO],
                                is_transpose=True,
                                start=False,
                                stop=True,
                            )
                            add_dep_helper(t2.ins, t1.ins, sync=False, reason="tpord")
                            nc.scalar.activation(
                                e_bf[:, jb, :], tp[:], Tanh, bias=wahT[:, hc, b : b + 1]
                            )
                        nc.tensor.matmul(
                            sc[:],
                            awT[:, hc : hc + 1],
                            e_bf[:],
                            start=(hc == 0),
                            stop=(hc == NHC - 1),
                        )

                    for jb in range(2):
                        b = b0 + jb
                        # exp with fused row-sum, then 1/sum
                        nc.scalar.activation(
                            expS[0:1, b * LP : b * LP + L],
                            sc[0:1, jb, :],
                            Exp,
                            accum_out=sums[0:1, b : b + 1],
                        )
                        nc.vector.reciprocal(
                            rsum[0:1, b : b + 1], sums[0:1, b : b + 1]
                        )

                        # alphaT columns via K=1 matmuls; rhs=1/sum normalizes
                        aT = pool_aT.tile([128, 2], F32)
                        if prev_aT_read is not None:
                            # bufs=1: this start=True reopens the bank; it must
                            # wait for the previous batch's col-1 read (regions
                            # are disjoint, so Tile tracks no dep itself)
                            pre_m1 = prev_aT_read
                        else:
                            pre_m1 = None
                        m1 = nc.tensor.matmul(
                            aT[:, 0:1],
                            expS[0:1, b * LP : b * LP + 128],
                            rsum[0:1, b : b + 1],
                            start=True,
                            stop=False,
                        )
                        m2 = nc.tensor.matmul(
                            aT[:, 1:2],
                            expS[0:1, b * LP + 128 : b * LP + 256],
                            rsum[0:1, b : b + 1],
                            start=False,
                            stop=True,
                        )
                        add_dep_helper(m2.ins, m1.ins, sync=False, reason="aTord")
                        if pre_m1 is not None:
                            add_dep_helper(m1.ins, pre_m1, sync=True, reason="aTwar")
                        c1 = nc.vector.tensor_copy(aT_sb[:, b, 0:1], aT[:, 0:1])
                        # col-0 read must wait until the accumulation group
                        # (closed by m2) is complete
                        add_dep_helper(c1.ins, m2.ins, sync=True, reason="aTgrp")
                        c2 = nc.vector.tensor_copy(
                            aT_sb[0:L_LO, b, 1:2], aT[0:L_LO, 1:2]
                        )
                        prev_aT_read = c2.ins

                        # -------- phase 2: weighted sum of att_feats --------
                        for q in range(NFQ):
                            fsl = slice(q * 512, (q + 1) * 512)
                            ao = pool_ao.tile([1, 512], F32)
                            nc.tensor.matmul(
                                ao[:],
                                aT_sb[:, b, 0:1],
                                af_hi[:, jb, fsl],
                                start=True,
                                stop=False,
                            )
                            nc.tensor.matmul(
                                ao[:],
                                aT_sb[0:L_LO, b, 1:2],
                                af_lo[:, jb, fsl],
                                start=False,
                                stop=True,
                            )
                            osl = slice(
                                jb * FEAT + q * 512, jb * FEAT + (q + 1) * 512
                            )
                            if q % 2 == 0:
                                nc.vector.tensor_copy(ob[0:1, osl], ao[:])
                            else:
                                nc.scalar.copy(ob[0:1, osl], ao[:])

                    nc.sync.dma_start(out_d[b0 : b0 + 2, :], ob[:])

    if split:
        _split_sync(nc)
    return nc


_NC_CACHE = None


def _get_nc():
    global _NC_CACHE
    if _NC_CACHE is None:
        _NC_CACHE = build_nc()
    return _NC_CACHE


def _make_in_maps(h, att_feats, p_att_feats, Wah_w, alpha_w):
    h = np.ascontiguousarray(h, dtype=np.float32)
    att_feats = np.ascontiguousarray(att_feats, dtype=np.float32)
    p_att_feats = np.ascontiguousarray(p_att_feats, dtype=np.float32)
    Wah_w = np.ascontiguousarray(Wah_w, dtype=np.float32)
    alpha_w = np.ascontiguousarray(alpha_w, dtype=np.float32)
    in_maps = []
    for i in range(NCORES):
        sl = slice(i * BL, (i + 1) * BL)
        in_maps.append(
            {
                "h": np.ascontiguousarray(h[sl]),
                "att_feats": np.ascontiguousarray(att_feats[sl]),
                "p_att_feats": np.ascontiguousarray(p_att_feats[sl]),
                "Wah_w": Wah_w,
                "alpha_w": alpha_w,
            }
        )
    return in_maps


def run_spmd(h, att_feats, p_att_feats, Wah_w, alpha_w, trace=False):
    """Run the SPMD kernel; returns (full_output, BassKernelResults)."""
    from concourse.bass_utils import run_bass_kernel_spmd

    nc = _get_nc()
    in_maps = _make_in_maps(h, att_feats, p_att_feats, Wah_w, alpha_w)
    res = run_bass_kernel_spmd(nc, in_maps, list(range(NCORES)), trace=trace)
    out = np.concatenate([res.results[i]["out"] for i in range(NCORES)], axis=0)
    return out, res


def kernel(h, att_feats, p_att_feats, Wah_w, alpha_w):
    out, _ = run_spmd(h, att_feats, p_att_feats, Wah_w, alpha_w, trace=False)
    return out


# revision 31
# speedup vs baseline: 1.1150x; 1.0485x over previous
"""Trainium2 Bass kernel for additive-attention pooling.

Computation (per batch row b):
    Wah   = h @ Wah_w.T                         [B, HID]
    e     = tanh(Wah[:, None, :] + p_att_feats) [B, L, HID]
    s     = e @ alpha_w[0]                      [B, L]
    alpha = softmax(s, -1)                      [B, L]
    att   = sum_l alpha[b, l] * att_feats[b, l, :]   [B, FEAT]

Sharding: pure data parallel over the batch dim, 32 rows per core on 8
NeuronCores; the small Wah_w / alpha_w weights are replicated.

Per-core dataflow (DMA-bound, ~64 MB of input per core):
  setup : PE-transpose h and Wah_w, compute WahT[h, b]; transpose alpha_w.
  phase1: stream p_att[b] -> PE transpose to [h, l] -> ScalarE fused
          bias(=Wah)+tanh (bf16 out) -> TensorE contracts h with alpha_w
          -> scores[1, L] -> ScalarE exp with fused row-sum -> reciprocal
          -> two K=1 matmuls transpose alpha row into a [L, 1] column,
          folding the 1/sum normalization into the matmul's rhs scalar.
  phase2: stream att_feats[b] as [l, f] tiles -> per batch 8 matvec
          matmuls (float32r, full-rate fp32 streaming) accumulate
          att[1, 512] per PSUM bank -> copy to staging -> one output DMA.
"""

import os
import sys
import types

sys.path.insert(0, "/opt/trn_rl_repo")

# This image's antenv package lacks axon_hooks; provide it so
# concourse.bass_utils can import it (trace path) without crashing.
if "antenv.axon_hooks" not in sys.modules:
    _m = types.ModuleType("antenv.axon_hooks")

    def _set_hook(h):
        _m._hook = h

    def _get_hook():
        return getattr(_m, "_hook", None)

    _m.set_axon_ntff_profile_hook = _set_hook
    _m.get_axon_ntff_profile_hook = _get_hook
    sys.modules["antenv.axon_hooks"] = _m
    import antenv

    antenv.axon_hooks = _m

import numpy as np  # noqa: E402
import bass_rust  # noqa: E402
import concourse.bass as bass  # noqa: E402
import concourse.tile as tile  # noqa: E402
from concourse import mybir  # noqa: E402
from concourse.masks import make_identity  # noqa: E402
from concourse.tile_rust import add_dep_helper  # noqa: E402

F32 = mybir.dt.float32
F32R = mybir.dt.float32r
BF16 = mybir.dt.bfloat16
PSUM = bass.MemorySpace.PSUM
Tanh = mybir.ActivationFunctionType.Tanh
Exp = mybir.ActivationFunctionType.Exp

B, L, RNN, HID, FEAT = 256, 196, 1024, 512, 2048
NCORES = 8
BL = B // NCORES  # batch rows per core
L_HI = 128
L_LO = L - L_HI  # 68
NHC = HID // 128  # h chunks
NRC = RNN // 128  # r chunks
NFQ = FEAT // 512  # psum-bank-sized f chunks

# float32r streams the rhs at 1 cycle/row (vs 4 for float32) when N >= 256.
USE_F32R = os.environ.get("KERNEL_NO_F32R", "") != "1"
AF_BUFS = int(os.environ.get("KERNEL_AF_BUFS", "3"))


def _split_sync(nc):
    """walrus in this image encodes at most ONE semaphore wait and ONE
    semaphore update per instruction; Tile freely emits several. Move the
    extras onto single-wait/single-update NoOp carriers on the same engine
    (engine queues are strict FIFO, so a preceding NoOp's wait gates the
    instruction and a following NoOp's update fires after it completes)."""
    dma_types = {
        "InstDMACopy",
        "InstTensorLoad",
        "InstTensorSave",
        "InstDmaTransposeAnt",
        "InstTensorCopy",
    }
    for f in nc.m.functions:
        for bb in f.blocks:
            new = []
            changed = False
            for ins in bb.instructions:
                si = ins.sync_info
                if si is None:
                    new.append(ins)
                    continue
                waits = list(si.on_wait)
                updates = list(si.on_update)
                if len(waits) <= 1 and len(updates) <= 1:
                    new.append(ins)
                    continue
                changed = True
                tname = type(ins).__name__
                for j, w in enumerate(waits[:-1]):
                    nop = mybir.InstNoOp(name=f"{ins.name}_w{j}", ins=[], outs=[])
                    nop.engine = ins.engine
                    nop.sync_info = bass_rust.SyncInfo(on_wait=[w], on_update=[])
                    new.append(nop)
                keep_w = waits[-1:]
                post_u = []
                keep_u = updates
                if len(updates) > 1:
                    if tname in dma_types:
                        raise RuntimeError(
                            f"DMA instruction {ins.name} carries {len(updates)} "
                            "sem updates; cannot split without changing semantics"
                        )
                    keep_u = updates[:1]
                    post_u = updates[1:]
                ins.sync_info = bass_rust.SyncInfo(on_wait=keep_w, on_update=keep_u)
                new.append(ins)
                for j, u in enumerate(post_u):
                    nop = mybir.InstNoOp(name=f"{ins.name}_u{j}", ins=[], outs=[])
                    nop.engine = ins.engine
                    nop.sync_info = bass_rust.SyncInfo(on_wait=[], on_update=[u])
                    new.append(nop)
            if changed:
                bb.instructions = new


def build_nc(split=True):
    nc = bass.Bass()
    mm_dt = F32R if USE_F32R else F32
    h_d = nc.declare_dram_parameter("h", [BL, RNN], F32, isOutput=False)
    af_d = nc.declare_dram_parameter("att_feats", [BL, L, FEAT], mm_dt, isOutput=False)
    pa_d = nc.declare_dram_parameter("p_att_feats", [BL, L, HID], F32, isOutput=False)
    ww_d = nc.declare_dram_parameter("Wah_w", [HID, RNN], F32, isOutput=False)
    aw_d = nc.declare_dram_parameter("alpha_w", [1, HID], F32, isOutput=False)
    out_d = nc.declare_dram_parameter("out", [BL, FEAT], F32, isOutput=True)

    with tile.TileContext(nc) as tc:
        with tc.tile_pool(name="singles", bufs=1) as singles:
            identity = singles.tile([128, 128], F32)
            make_identity(nc, identity[:])
            wahT = singles.tile([128, NHC, BL], F32)  # WahT[h % 128, hc, b]
            awT = singles.tile([128, NHC], BF16)  # alpha_w^T chunks
            # exp(scores): 256-wide zero-padded slot per batch so the two
            # alphaT transpose matmuls both span 128 output partitions (the
            # PSUM accumulation-group bookkeeping is per-partition)
            LP = 256
            expS = singles.tile([1, BL * LP], F32)
            nc.gpsimd.memset(expS[:], 0.0)
            sums = singles.tile([1, BL], F32)
            rsum = singles.tile([1, BL], F32)
            aT_sb = singles.tile([128, BL, 2], mm_dt)  # alphaT cols (hi, lo)

            # ---------------- setup: weights ----------------
            with (
                tc.tile_pool(name="setup_sb", bufs=1) as ssb,
                tc.tile_pool(name="setup_ps", bufs=2, space=PSUM) as sps,
            ):
                h_sb = ssb.tile([BL, RNN], F32)
                nc.sync.dma_start(h_sb[:], h_d[:])
                ww_sb = ssb.tile([128, NHC, RNN], F32)
                nc.sync.dma_start(
                    ww_sb[:], ww_d[:].rearrange("(c p) r -> p c r", p=128)
                )
                aw_sb = ssb.tile([1, HID], F32)
                nc.sync.dma_start(aw_sb[:], aw_d[:])
                ones11 = ssb.tile([1, 1], F32)
                nc.gpsimd.memset(ones11[:], 1.0)

                # h^T: [r % 128, rc, b]
                hT = ssb.tile([128, NRC, BL], F32)
                for rc in range(NRC):
                    ps = sps.tile([128, BL], F32, tag="t_small")
                    nc.tensor.transpose(
                        ps[:], h_sb[:, rc * 128 : (rc + 1) * 128], identity[:BL, :BL]
                    )
                    nc.vector.tensor_copy(hT[:, rc, :], ps[:])

                # Wah_w^T: [r % 128, rc, h]
                wwT = ssb.tile([128, NRC, HID], F32)
                for rc in range(NRC):
                    for hc in range(NHC):
                        ps = sps.tile([128, 128], F32, tag="t_big")
                        nc.tensor.transpose(
                            ps[:],
                            ww_sb[:, hc, rc * 128 : (rc + 1) * 128],
                            identity[:],
                        )
                        nc.vector.tensor_copy(
                            wwT[:, rc, hc * 128 : (hc + 1) * 128], ps[:]
                        )

                # WahT[h, b] = sum_r Wah_w[h, r] * h[b, r]
                for hc in range(NHC):
                    ps = sps.tile([128, BL], F32, tag="mm")
                    for rc in range(NRC):
                        nc.tensor.matmul(
                            ps[:],
                            wwT[:, rc, hc * 128 : (hc + 1) * 128],
                            hT[:, rc, :],
                            start=(rc == 0),
                            stop=(rc == NRC - 1),
                        )
                    nc.vector.tensor_copy(wahT[:, hc, :], ps[:])

                # alpha_w^T columns (bf16 to match bf16 e tiles)
                for hc in range(NHC):
                    ps = sps.tile([128, 1], F32, tag="aw")
                    nc.tensor.matmul(
                        ps[:],
                        aw_sb[0:1, hc * 128 : (hc + 1) * 128],
                        ones11[:],
                        start=True,
                        stop=True,
                    )
                    nc.vector.tensor_copy(awT[:, hc : hc + 1], ps[:])

            # ---------------- streaming batch loop ----------------
            with (
                tc.tile_pool(name="pa", bufs=3) as pool_pa,
                tc.tile_pool(name="af", bufs=AF_BUFS) as pool_af,
                tc.tile_pool(name="e", bufs=6) as pool_e,
                tc.tile_pool(name="tp_ps", bufs=3, space=PSUM) as pool_tp,
                tc.tile_pool(name="sc_ps", bufs=1, space=PSUM) as pool_sc,
                tc.tile_pool(name="aT_ps", bufs=1, space=PSUM) as pool_aT,
                tc.tile_pool(name="ao_ps", bufs=3, space=PSUM) as pool_ao,
                tc.tile_pool(name="ob", bufs=2) as pool_ob,
            ):
                prev_aT_read = None
                for p in range(BL // 2):
                    b0 = 2 * p
                    pa_hi = pool_pa.tile([L_HI, 2, HID], F32, tag="pa_hi")
                    nc.sync.dma_start(
                        pa_hi[:],
                        pa_d[b0 : b0 + 2, 0:L_HI, :].rearrange("b l c -> l b c"),
                    )
                    # 68-partition DMAs fan out to only 4 SDMA engines (engine
                    # count = largest power of two dividing P, capped at 16);
                    # split 68 -> 64 + 4 to spread across all 16 engines.
                    pa_lo = pool_pa.tile([L_LO, 2, HID], F32, tag="pa_lo")
                    nc.sync.dma_start(
                        pa_lo[0:64],
                        pa_d[b0 : b0 + 2, L_HI : L_HI + 64, :].rearrange(
                            "b l c -> l b c"
                        ),
                    )
                    nc.sync.dma_start(
                        pa_lo[64:L_LO],
                        pa_d[b0 : b0 + 2, L_HI + 64 : L, :].rearrange(
                            "b l c -> l b c"
                        ),
                    )
                    af_hi = pool_af.tile([L_HI, 2, FEAT], mm_dt, tag="af_hi")
                    af_lo = pool_af.tile([L_LO, 2, FEAT], mm_dt, tag="af_lo")
                    nc.sync.dma_start(
                        af_hi[:],
                        af_d[b0 : b0 + 2, 0:L_HI, :].rearrange("b l c -> l b c"),
                    )
                    nc.sync.dma_start(
                        af_lo[0:64],
                        af_d[b0 : b0 + 2, L_HI : L_HI + 64, :].rearrange(
                            "b l c -> l b c"
                        ),
                    )
                    nc.sync.dma_start(
                        af_lo[64:L_LO],
                        af_d[b0 : b0 + 2, L_HI + 64 : L, :].rearrange(
                            "b l c -> l b c"
                        ),
                    )

                    # output rows for this pair, staged flat on partition 0
                    ob = pool_ob.tile([1, 2 * FEAT], F32)

                    # -------- phase 1: scores for both batches of the pair --------
                    sc = pool_sc.tile([1, 2, L], F32)
                    for hc in range(NHC):
                        hsl = slice(hc * 128, (hc + 1) * 128)
                        e_bf = pool_e.tile([128, 2, L], BF16)
                        for jb in range(2):
                            b = b0 + jb
                            tp = pool_tp.tile([128, L], F32)
                            t1 = nc.tensor.matmul(
                                tp[:, 0:L_HI],
                                pa_hi[:, jb, hsl],
                                identity[:],
                                is_transpose=True,
                                start=True,
                                stop=False,
                            )
                            t2 = nc.tensor.matmul(
                                tp[:, L_HI:L],
                                pa_lo[:, jb, hsl],
                                identity[:L_LO, :L_LO],
                                is_transpose=True,
                                start=False,
                                stop=True,
                            )
                            add_dep_helper(t2.ins, t1.ins, sync=False, reason="tpord")
                            nc.scalar.activation(
                                e_bf[:, jb, :], tp[:], Tanh, bias=wahT[:, hc, b : b + 1]
                            )
                        nc.tensor.matmul(
                            sc[:],
                            awT[:, hc : hc + 1],
                            e_bf[:],
                            start=(hc == 0),
                            stop=(hc == NHC - 1),
                        )

                    for jb in range(2):
                        b = b0 + jb
                        # exp with fused row-sum, then 1/sum
                        nc.scalar.activation(
                            expS[0:1, b * LP : b * LP + L],
                            sc[0:1, jb, :],
                            Exp,
                            accum_out=sums[0:1, b : b + 1],
                        )
                        nc.vector.reciprocal(
                            rsum[0:1, b : b + 1], sums[0:1, b : b + 1]
                        )

                        # alphaT columns via K=1 matmuls; rhs=1/sum normalizes
                        aT = pool_aT.tile([128, 2], F32)
                        if prev_aT_read is not None:
                            # bufs=1: this start=True reopens the bank; it must
                            # wait for the previous batch's col-1 read (regions
                            # are disjoint, so Tile tracks no dep itself)
                            pre_m1 = prev_aT_read
                        else:
                            pre_m1 = None
                        m1 = nc.tensor.matmul(
                            aT[:, 0:1],
                            expS[0:1, b * LP : b * LP + 128],
                            rsum[0:1, b : b + 1],
                            start=True,
                            stop=False,
                        )
                        m2 = nc.tensor.matmul(
                            aT[:, 1:2],
                            expS[0:1, b * LP + 128 : b * LP + 256],
                            rsum[0:1, b : b + 1],
                            start=False,
                            stop=True,
                        )
                        add_dep_helper(m2.ins, m1.ins, sync=False, reason="aTord")
                        if pre_m1 is not None:
                            add_dep_helper(m1.ins, pre_m1, sync=True, reason="aTwar")
                        c1 = nc.vector.tensor_copy(aT_sb[:, b, 0:1], aT[:, 0:1])
                        # col-0 read must wait until the accumulation group
                        # (closed by m2) is complete
                        add_dep_helper(c1.ins, m2.ins, sync=True, reason="aTgrp")
                        c2 = nc.vector.tensor_copy(
                            aT_sb[0:L_LO, b, 1:2], aT[0:L_LO, 1:2]
                        )
                        prev_aT_read = c2.ins

                        # -------- phase 2: weighted sum of att_feats --------
                        for q in range(NFQ):
                            fsl = slice(q * 512, (q + 1) * 512)
                            ao = pool_ao.tile([1, 512], F32)
                            nc.tensor.matmul(
                                ao[:],
                                aT_sb[:, b, 0:1],
                                af_hi[:, jb, fsl],
                                start=True,
                                stop=False,
                            )
                            nc.tensor.matmul(
                                ao[:],
                                aT_sb[0:L_LO, b, 1:2],
                                af_lo[:, jb, fsl],
                                start=False,
                                stop=True,
                            )
                            osl = slice(
                                jb * FEAT + q * 512, jb * FEAT + (q + 1) * 512
                            )
                            if q % 2 == 0:
                                nc.vector.tensor_copy(ob[0:1, osl], ao[:])
                            else:
                                nc.scalar.copy(ob[0:1, osl], ao[:])

                    nc.sync.dma_start(out_d[b0 : b0 + 2, :], ob[:])

    if split:
        _split_sync(nc)
    return nc


_NC_CACHE = None


def _get_nc():
    global _NC_CACHE
    if _NC_CACHE is None:
        _NC_CACHE = build_nc()
    return _NC_CACHE


def _make_in_maps(h, att_feats, p_att_feats, Wah_w, alpha_w):
    h = np.ascontiguousarray(h, dtype=np.float32)
    att_feats = np.ascontiguousarray(att_feats, dtype=np.float32)
    p_att_feats = np.ascontiguousarray(p_att_feats, dtype=np.float32)
    Wah_w = np.ascontiguousarray(Wah_w, dtype=np.float32)
    alpha_w = np.ascontiguousarray(alpha_w, dtype=np.float32)
    in_maps = []
    for i in range(NCORES):
        sl = slice(i * BL, (i + 1) * BL)
        in_maps.append(
            {
                "h": np.ascontiguousarray(h[sl]),
                "att_feats": np.ascontiguousarray(att_feats[sl]),
                "p_att_feats": np.ascontiguousarray(p_att_feats[sl]),
                "Wah_w": Wah_w,
                "alpha_w": alpha_w,
            }
        )
    return in_maps


def run_spmd(h, att_feats, p_att_feats, Wah_w, alpha_w, trace=False):
    """Run the SPMD kernel; returns (full_output, BassKernelResults)."""
    from concourse.bass_utils import run_bass_kernel_spmd

    nc = _get_nc()
    in_maps = _make_in_maps(h, att_feats, p_att_feats, Wah_w, alpha_w)
    res = run_bass_kernel_spmd(nc, in_maps, list(range(NCORES)), trace=trace)
    out = np.concatenate([res.results[i]["out"] for i in range(NCORES)], axis=0)
    return out, res


def kernel(h, att_feats, p_att_feats, Wah_w, alpha_w):
    out, _ = run_spmd(h, att_feats, p_att_feats, Wah_w, alpha_w, trace=False)
    return out
